# revision 15
# baseline (speedup 1.0000x reference)
"""Trainium2 Bass kernel for the DependencyParseModel problem.

Pipeline (replicated biLSTM, pairwise scoring sharded over 8 cores):
  1. host: embedding gather, weight permute into PE-friendly layouts.
     Everything uses a uniform 100-wide chunking (400 = 4x100 hidden,
     1600 = 16x100 gates) so no instruction ever loads padding: the
     LDWEIGHTS pipeline cadence has a flat ~85ns floor, so 16 chunks of
     100 columns beat 16 chunks of 128 (padded).
  2. device: XW = Wih @ x_aug (dense matmuls, bias via an extra ones
     contract row), 256-step LSTM scans with weights-stationary matvecs
     (bf16), both directions interleaved. Gate groups [i,f,g~] and [o]
     use separate full-bank PSUM groups so the sigmoid fires before the
     o-gate matvecs finish and each direction's pointwise chain hides
     under the other direction's matvec block.
  3. device: pairwise-MLP scoring in transposed layout: bias-add on DVE,
     tanh on the scalar engine, W2 contraction on the PE.
  4. host: assemble [256,256], add b2, zero diagonal

Self-contained: hardcodes all shapes; no sibling imports.
"""

import os
import numpy as np
import ml_dtypes

T = 256
H = 400            # LSTM hidden
C = 100            # chunk width (H = 4*C, gates = 16*C)
G = 1600           # gate dim (4 gates x 400), unpadded
MLP = 400
NCORES = 8
RPC = T // NCORES  # rows per core (head-word rows)

BF16 = ml_dtypes.bfloat16

# torch gate order i,f,g,o -> our column-chunk order [i, f, g~, o];
# chunk c = 4*tau + d//100 holds gates d in [100*(c%4), 100*(c%4)+100)
_TAU_SRC = [(0, 400), (400, 800), (800, 1200), (1200, 1600)]  # i, f, g~, o


def _gate_relayout(W):
    """[1600, K] torch-order -> [1600, K] chunk-order, g~ block doubled."""
    K = W.shape[1] if W.ndim == 2 else 1
    W2d = W.reshape(1600, -1)
    out = np.empty((16, C, W2d.shape[1]), np.float32)
    for tau, (s, e) in enumerate(_TAU_SRC):
        blk = W2d[s:e].reshape(4, C, -1)
        if tau == 2:
            blk = blk * 2.0          # tanh(x) = 2*sigmoid(2x) - 1
        out[4 * tau: 4 * tau + 4] = blk
    return out.reshape(1600, -1) if W.ndim == 2 else out.reshape(1600)


def _h_tile(v):
    """[400] -> [100, 4] tile, elem d -> (d % 100, d // 100)."""
    return np.ascontiguousarray(np.asarray(v, np.float32).reshape(4, C).T)


_PROG_CACHE = {}


def _get_program(n_steps=T):
    key = ("prog", n_steps)
    if key in _PROG_CACHE:
        return _PROG_CACHE[key]

    import concourse.bass as bass
    import concourse.mybir as mybir
    import concourse.tile as tile
    from concourse import bacc

    F32 = mybir.dt.float32
    BF = mybir.dt.bfloat16
    AF = mybir.ActivationFunctionType
    ALU = mybir.AluOpType

    nc = bacc.Bacc("TRN2", target_bir_lowering=False, debug=False,
                   enable_asserts=False, num_devices=NCORES)

    # ---- DRAM I/O ----
    d_xT = nc.dram_tensor("xT", [H + 1, T], BF, kind="ExternalInput").ap()
    d_wihT0 = [nc.dram_tensor(f"wihT0{d}", [H + 1, G], BF, kind="ExternalInput").ap() for d in "fb"]
    d_wihT1 = [nc.dram_tensor(f"wihT1{d}", [2 * H + 1, G], BF, kind="ExternalInput").ap() for d in "fb"]
    d_whhT = [[nc.dram_tensor(f"whhT{l}{d}", [H, G], BF, kind="ExternalInput").ap() for d in "fb"]
              for l in (0, 1)]
    d_h0 = [[nc.dram_tensor(f"h0_{l}{d}", [C, 4], BF, kind="ExternalInput").ap() for d in "fb"]
            for l in (0, 1)]
    d_c0 = [[nc.dram_tensor(f"c0_{l}{d}", [C, 4], F32, kind="ExternalInput").ap() for d in "fb"]
            for l in (0, 1)]
    d_w1aT = nc.dram_tensor("w1aT", [2 * H + 1, MLP], BF, kind="ExternalInput").ap()
    d_w1bT = nc.dram_tensor("w1bT", [2 * H + 1, MLP], BF, kind="ExternalInput").ap()
    d_skT = nc.dram_tensor("skT", [T, RPC], BF, kind="ExternalInput").ap()
    d_w2T = nc.dram_tensor("w2T", [C, 4], BF, kind="ExternalInput").ap()
    d_ident = nc.dram_tensor("ident", [128, 128], BF, kind="ExternalInput").ap()
    d_out = nc.dram_tensor("scores_t", [T, RPC], F32, kind="ExternalOutput").ap()
    DBG = bool(int(os.environ.get("KERNEL_DEBUG", "0")))
    if DBG:
        d_dbg = [[nc.dram_tensor(f"dbg_hh{l}{d}", [C, 4 * T], F32,
                                 kind="ExternalOutput").ap() for d in (0, 1)]
                 for l in (0, 1)]

    with tile.TileContext(nc) as tc:
        from contextlib import ExitStack
        with ExitStack() as ctx:
            const = ctx.enter_context(tc.tile_pool(name="const", bufs=1))
            state = ctx.enter_context(tc.tile_pool(name="state", bufs=1))
            whhp = ctx.enter_context(tc.tile_pool(name="whhp", bufs=1))

            # --- constants / initial state in SBUF ---
            xT_sb = []
            for kc in range(4):
                xt = const.tile([C, T], BF, name=f"xT{kc}")
                nc.sync.dma_start(xt, d_xT[C * kc:C * (kc + 1), :])
                xT_sb.append(xt)
            xones = const.tile([1, T], BF, name="xones")
            nc.sync.dma_start(xones, d_xT[H:H + 1, :])
            h0_sb = [[None, None], [None, None]]
            c_sb = [[None, None], [None, None]]
            for l in (0, 1):
                for d in (0, 1):
                    t0 = const.tile([C, 4], BF, name=f"h0sb{l}{d}")
                    nc.sync.dma_start(t0, d_h0[l][d])
                    h0_sb[l][d] = t0
                    t1 = state.tile([C, 4], F32, name=f"csb{l}{d}")
                    nc.sync.dma_start(t1, d_c0[l][d])
                    c_sb[l][d] = t1
            whh_sb = [[None, None], [None, None]]
            for l in (0, 1):
                for d in (0, 1):
                    chunks = []
                    for kc in range(4):
                        w = whhp.tile([C, G], BF, name=f"whh{l}{d}{kc}")
                        nc.sync.dma_start(w, d_whhT[l][d][C * kc:C * (kc + 1), :])
                        chunks.append(w)
                    whh_sb[l][d] = chunks
            w1aT_sb, w1bT_sb = [], []
            for kc in range(8):
                wa = const.tile([C, MLP], BF, name=f"w1aT{kc}")
                nc.sync.dma_start(wa, d_w1aT[C * kc:C * (kc + 1), :])
                w1aT_sb.append(wa)
                wb = const.tile([C, MLP], BF, name=f"w1bT{kc}")
                nc.sync.dma_start(wb, d_w1bT[C * kc:C * (kc + 1), :])
                w1bT_sb.append(wb)
            w1a_bias = const.tile([1, MLP], BF, name="w1abias")
            nc.sync.dma_start(w1a_bias, d_w1aT[2 * H:2 * H + 1, :])
            skT_sb = []
            for kc in range(2):
                sk = const.tile([128, RPC], BF, name=f"skT{kc}")
                nc.sync.dma_start(sk, d_skT[128 * kc:128 * (kc + 1), :])
                skT_sb.append(sk)
            w2T_sb = const.tile([C, 4], BF, name="w2T")
            nc.sync.dma_start(w2T_sb, d_w2T)
            ident_sb = const.tile([128, 128], BF, name="ident")
            nc.sync.dma_start(ident_sb, d_ident)

            # persistent per-(layer,dir) h history [100, 4*T] bf16, col 4t+c = h_t[100c+p]
            hh_sb = [[state.tile([C, 4 * T], BF, name=f"hh{l}{d}") for d in (0, 1)]
                     for l in (0, 1)]
            # XW^T buffers, reused across layers: [100, 16*T] bf16, col m*T + t
            xwt_sb = [state.tile([C, 16 * T], BF, name=f"xwt{d}") for d in (0, 1)]

            def xwt_phase(layer, preloaded=None):
                """xwt_sb[d] <- Wih[layer][d] @ x_aug (all timesteps)."""
                K = 4 if layer == 0 else 8
                d_wih = d_wihT0 if layer == 0 else d_wihT1
                with tc.tile_pool(name=f"wihp{layer}", bufs=1) as wp, \
                     tc.tile_pool(name=f"xwps{layer}", bufs=4, space="PSUM") as pp:
                    for d in (0, 1):
                        if preloaded is not None:
                            wih_sb, wih_bias = preloaded[d]
                        else:
                            wih_sb = []
                            for kc in range(K):
                                w = wp.tile([C, G], BF, name=f"wih{layer}{d}{kc}",
                                            tag=f"wih{kc}")
                                nc.sync.dma_start(w, d_wih[d][C * kc:C * (kc + 1), :])
                                wih_sb.append(w)
                            wih_bias = wp.tile([1, G], BF, name=f"wihb{layer}{d}",
                                               tag="wihb")
                            nc.sync.dma_start(wih_bias, d_wih[d][K * C:K * C + 1, :])
                        if layer == 0:
                            rhs = xT_sb
                        else:
                            rhs = []
                            for kc in range(K):
                                hhr = hh_sb[0][kc // 4][:].rearrange(
                                    "p (t c) -> p c t", c=4)
                                rhs.append(hhr[:, kc % 4, :])
                        for m in range(16):
                            ps = pp.tile([C, T], F32, name=f"xwps{layer}{d}{m}",
                                         tag="xwps")
                            for kc in range(K):
                                nc.tensor.matmul(
                                    ps, wih_sb[kc][:, C * m:C * (m + 1)], rhs[kc],
                                    start=(kc == 0), stop=False)
                            nc.tensor.matmul(
                                ps, wih_bias[:, C * m:C * (m + 1)], xones,
                                start=False, stop=True)
                            nc.vector.tensor_copy(
                                xwt_sb[d][:, T * m:T * (m + 1)], ps)

            def scan_phase(layer):
                # gate chunk layout: cols 0:4=i, 4:8=f, 8:12=g~, 12:16=o with
                # g~ pre-activations DOUBLED (host scaled the weights), so one
                # sigmoid gives tanh(x) = 2*(sigmoid(2x)-0.5) for the g~ block.
                # [i,f,g~] and [o] are separate PSUM accumulation groups (own
                # banks) so the sigmoid fires before the o matvecs finish.
                with tc.tile_pool(name=f"psg{layer}", bufs=2, space="PSUM") as p_g, \
                     tc.tile_pool(name=f"sg{layer}", bufs=3) as sgp:
                    for s in range(n_steps):
                        for d in (0, 1):
                            t = s if d == 0 else T - 1 - s
                            hh = hh_sb[layer][d]
                            if s == 0:
                                h_prev = h0_sb[layer][d]
                            else:
                                tp = t - 1 if d == 0 else t + 1
                                h_prev = hh[:, 4 * tp:4 * tp + 4]
                            xwr = xwt_sb[d][:].rearrange("p (m t) -> p m t", t=T)
                            g_ifg = p_g.tile([C, 12], F32, name=f"gifg{d}",
                                             tag=f"gifg{d}", padded_shape=[128, 512])
                            g_o = p_g.tile([C, 4], F32, name=f"go{d}",
                                           tag=f"go{d}", padded_shape=[128, 512])
                            nc.tensor.matmul(g_ifg, ident_sb[0:C, 0:C],
                                             xwr[:, 0:12, t],
                                             start=True, stop=False,
                                             skip_group_check=True)
                            for m in range(12):
                                for kc in range(4):
                                    nc.tensor.matmul(
                                        g_ifg[:, m:m + 1],
                                        whh_sb[layer][d][kc][:, C * m:C * (m + 1)],
                                        h_prev[:, kc:kc + 1],
                                        start=False,
                                        stop=(m == 11 and kc == 3),
                                        skip_group_check=True)
                            nc.tensor.matmul(g_o, ident_sb[0:C, 0:C],
                                             xwr[:, 12:16, t],
                                             start=True, stop=False,
                                             skip_group_check=True)
                            for m in range(12, 16):
                                for kc in range(4):
                                    nc.tensor.matmul(
                                        g_o[:, m - 12:m - 11],
                                        whh_sb[layer][d][kc][:, C * m:C * (m + 1)],
                                        h_prev[:, kc:kc + 1],
                                        start=False,
                                        stop=(m == 15 and kc == 3),
                                        skip_group_check=True)
                            S = sgp.tile([C, 12], F32, name=f"S{d}", tag=f"S{d}")
                            So = sgp.tile([C, 4], F32, name=f"So{d}", tag=f"So{d}")
                            nc.scalar.activation(S, g_ifg, AF.Sigmoid)
                            nc.scalar.activation(So, g_o, AF.Sigmoid)
                            cc = c_sb[layer][d]
                            t1 = sgp.tile([C, 4], F32, name=f"t1{d}", tag=f"t1{d}")
                            u2 = sgp.tile([C, 4], F32, name=f"u2{d}", tag=f"u2{d}")
                            # c = sig(f)*c + sig(i)*2*(S_g~ - 0.5)
                            nc.vector.tensor_mul(t1, S[:, 4:8], cc)
                            nc.vector.scalar_tensor_tensor(
                                u2, S[:, 8:12], -0.5, S[:, 0:4],
                                op0=ALU.add, op1=ALU.mult)
                            nc.vector.scalar_tensor_tensor(
                                cc, u2, 2.0, t1,
                                op0=ALU.mult, op1=ALU.add)
                            tct = sgp.tile([C, 4], F32, name=f"tc{d}", tag=f"tc{d}")
                            nc.scalar.activation(tct, cc, AF.Tanh)
                            nc.vector.tensor_mul(hh[:, 4 * t:4 * t + 4], So, tct)

            xwt_phase(0)
            # prefetch layer-1 Wih during the L0 scan (DMAs have no deps on
            # the scan, so the scheduler overlaps them with it)
            wih1_pre = [None, None]
            with tc.tile_pool(name="wihpre1", bufs=1) as wpre:
                for d in (0, 1):
                    chunks = []
                    for kc in range(8):
                        w = wpre.tile([C, G], BF, name=f"wihpre{d}{kc}")
                        nc.sync.dma_start(w, d_wihT1[d][C * kc:C * (kc + 1), :])
                        chunks.append(w)
                    wb = wpre.tile([1, G], BF, name=f"wihpreb{d}")
                    nc.sync.dma_start(wb, d_wihT1[d][2 * H:2 * H + 1, :])
                    wih1_pre[d] = (chunks, wb)
                scan_phase(0)
                xwt_phase(1, preloaded=wih1_pre)
            scan_phase(1)
            if DBG:
                with tc.tile_pool(name="dbgp", bufs=1) as dbgp:
                    for l in (0, 1):
                        for d in (0, 1):
                            dt_ = dbgp.tile([C, 4 * T], F32, name=f"dbg{l}{d}")
                            nc.vector.tensor_copy(dt_, hh_sb[l][d])
                            nc.sync.dma_start(d_dbg[l][d], dt_)

            # ---------- pairwise scoring (transposed layout) ----------
            def hvecT_chunk(kc, jt):
                """[100, 128]: hvec.T feats [100kc:+100], cols [128jt:+128]."""
                hhr = hh_sb[1][kc // 4][:].rearrange("p (t c) -> p c t", c=4)
                return hhr[:, kc % 4, 128 * jt:128 * (jt + 1)]

            def hvecT_full(kc):
                hhr = hh_sb[1][kc // 4][:].rearrange("p (t c) -> p c t", c=4)
                return hhr[:, kc % 4, :]

            with tc.tile_pool(name="pw", bufs=1) as pw:
                pjT_sb, piT_loc = [], []
                with tc.tile_pool(name="pwps", bufs=2, space="PSUM") as pwps:
                    # pjT[mc] = (W1b @ hvec.T) chunk: [100 mlp, 256 j]
                    for mc in range(4):
                        ps = pwps.tile([C, T], F32, name=f"pjTps{mc}", tag="projps")
                        for kc in range(8):
                            nc.tensor.matmul(ps,
                                             w1bT_sb[kc][:, C * mc:C * (mc + 1)],
                                             hvecT_full(kc),
                                             start=(kc == 0), stop=(kc == 7))
                        pj = pw.tile([C, T], BF, name=f"pjT{mc}")
                        nc.vector.tensor_copy(pj, ps)
                        pjT_sb.append(pj)
                    # pi rows: [128 i, 400 mlp] x2 (bias via ones contract row),
                    # then select this core's 32 rows
                    pi_sb = []
                    for jt in range(2):
                        ps = pwps.tile([128, MLP], F32, name=f"pips{jt}",
                                       tag="projps2")
                        for kc in range(8):
                            nc.tensor.matmul(ps, hvecT_chunk(kc, jt), w1aT_sb[kc],
                                             start=(kc == 0), stop=False)
                        nc.tensor.matmul(ps, xones[:, 128 * jt:128 * (jt + 1)],
                                         w1a_bias, start=False, stop=True)
                        pi = pw.tile([128, MLP], BF, name=f"pi{jt}")
                        nc.vector.tensor_copy(pi, ps)
                        pi_sb.append(pi)
                    ps = pwps.tile([RPC, MLP], F32, name="pikps", tag="projps2")
                    for kc in range(2):
                        nc.tensor.matmul(ps, skT_sb[kc], pi_sb[kc],
                                         start=(kc == 0), stop=(kc == 1))
                    pik = pw.tile([RPC, MLP], BF, name="pik")
                    nc.vector.tensor_copy(pik, ps)
                    # transpose pik -> piT_loc[mc]: [100 mlp, 32 rows] f32 sbuf
                    for mc in range(4):
                        tps = pwps.tile([C, RPC], BF, name=f"piTps{mc}",
                                        tag="projps")
                        nc.tensor.transpose(tps, pik[:, C * mc:C * (mc + 1)],
                                            ident_sb[0:RPC, 0:RPC])
                        tl = pw.tile([C, RPC], F32, name=f"piT{mc}")
                        nc.vector.tensor_copy(tl, tps)
                        piT_loc.append(tl)

                with tc.tile_pool(name="bps", bufs=1, space="PSUM") as bps, \
                     tc.tile_pool(name="bsb", bufs=3) as bsb:
                    scoresT_ps = [bps.tile([128, RPC], F32, name=f"scT{jh}",
                                           tag=f"scT{jh}") for jh in range(2)]
                    for g in range(8):          # 4 head rows per group
                        Tas = []
                        for mc in range(4):
                            Bt = bsb.tile([C, 4 * T], BF, name=f"B{g}{mc}",
                                          tag="B")
                            for q in range(4):
                                r = 4 * g + q
                                nc.vector.tensor_scalar_add(
                                    Bt[:, T * q:T * (q + 1)], pjT_sb[mc],
                                    piT_loc[mc][:, r:r + 1])
                            Ta = bsb.tile([C, 4 * T], BF, name=f"Ta{g}{mc}",
                                          tag=f"Ta{mc}", bufs=2)
                            nc.scalar.activation(Ta, Bt, AF.Tanh)
                            Tas.append(Ta)
                        # each psum column's 4-matmul accumulation group is
                        # consecutive (interleaved per-column groups misaccumulate)
                        for q in range(4):
                            r = 4 * g + q
                            for jh in range(2):
                                for mc in range(4):
                                    nc.tensor.matmul(
                                        scoresT_ps[jh][:, r:r + 1],
                                        Tas[mc][:, T * q + 128 * jh:T * q + 128 * (jh + 1)],
                                        w2T_sb[:, mc:mc + 1],
                                        start=(mc == 0), stop=(mc == 3),
                                        skip_group_check=True)
                    for jh in range(2):
                        sc = pw.tile([128, RPC], F32, name=f"scsb{jh}")
                        nc.vector.tensor_copy(sc, scoresT_ps[jh])
                        nc.sync.dma_start(d_out[128 * jh:128 * (jh + 1), :], sc)

    nc.compile()
    _PROG_CACHE[key] = nc
    return nc


def _prep_inputs(inputs):
    """Host-side prep: gather embeddings, build permuted device tensors."""
    I = {k: np.asarray(v) for k, v in inputs.items()}
    x = np.concatenate([I["word_emb"][I["words"]], I["tag_emb"][I["tags"]]],
                       axis=1).astype(np.float32)          # [T, 400]
    xT = np.ones((H + 1, T), np.float32)
    xT[:H] = x.T                                           # row 400 = ones (bias)

    common = {"xT": xT.astype(BF16)}
    for l in (0, 1):
        Din = H if l == 0 else 2 * H
        for di, d in enumerate("fb"):
            wih = _gate_relayout(I[f"Wih{l}"][di])          # [1600, Din]
            bias = _gate_relayout(I[f"bih{l}"][di] + I[f"bhh{l}"][di])  # [1600]
            wihp = np.concatenate([wih.T, bias[None, :]], axis=0)  # [Din+1, 1600]
            common[f"wihT{l}{d}"] = np.ascontiguousarray(wihp).astype(BF16)

            whh = _gate_relayout(I[f"Whh{l}"][di])          # [1600, 400]
            common[f"whhT{l}{d}"] = np.ascontiguousarray(whh.T).astype(BF16)

            common[f"h0_{l}{d}"] = _h_tile(I["h0"][l, di]).astype(BF16)
            common[f"c0_{l}{d}"] = _h_tile(I["c0"][l, di]).astype(np.float32)

    W1 = I["W1"].astype(np.float32)                         # [400, 1600]
    W1a, W1b = W1[:, :2 * H], W1[:, 2 * H:]                 # [400, 800] each

    def mlp_T(W, bias):
        return np.ascontiguousarray(
            np.concatenate([W.T, bias[None, :]], axis=0)).astype(BF16)  # [801, 400]

    common["w1aT"] = mlp_T(W1a, I["b1"].astype(np.float32))
    common["w1bT"] = mlp_T(W1b, np.zeros(MLP, np.float32))
    w2T = np.zeros((C, 4), np.float32)
    for mc in range(4):
        w2T[:, mc] = I["W2"][0][C * mc:C * (mc + 1)]
    common["w2T"] = w2T.astype(BF16)
    common["ident"] = np.eye(128, dtype=np.float32).astype(BF16)

    in_maps = []
    for k in range(NCORES):
        m = dict(common)
        sk = np.zeros((T, RPC), np.float32)
        sk[RPC * k + np.arange(RPC), np.arange(RPC)] = 1.0
        m["skT"] = sk.astype(BF16)
        in_maps.append(m)
    return in_maps, I


def _ensure_ntff_hook():
    """Shim antenv.axon_hooks (absent in this image) so trace=True works."""
    import sys
    import types
    import antenv
    if hasattr(antenv, "axon_hooks") or "antenv.axon_hooks" in sys.modules:
        return
    hook = None
    try:
        from trn_agent_boot.trn_boot import _ntff_profile_via_ctypes
        hook = _ntff_profile_via_ctypes("/opt/axon/libaxon_pjrt.so")
    except Exception:
        hook = None
    mod = types.ModuleType("antenv.axon_hooks")
    state = {"hook": hook}
    mod.get_axon_ntff_profile_hook = lambda: state["hook"]
    mod.set_axon_ntff_profile_hook = lambda h: state.update(hook=h)
    sys.modules["antenv.axon_hooks"] = mod
    antenv.axon_hooks = mod


def kernel(**inputs):
    from concourse import bass_utils
    from concourse.bass_interp import get_hw_module

    nc = _get_program()
    in_maps, I = _prep_inputs(inputs)

    trace = bool(int(os.environ.get("KERNEL_TRACE", "0")))
    if trace:
        _ensure_ntff_hook()
    old_m = nc.m
    nc.m = get_hw_module(nc.m)
    try:
        res = bass_utils.run_bass_kernel_spmd(
            nc, in_maps, core_ids=list(range(NCORES)), trace=trace)
    finally:
        nc.m = old_m
    if trace and res.exec_time_ns is not None:
        print(f"HW exec time: {res.exec_time_ns} ns")
        kernel.last_exec_time_ns = res.exec_time_ns

    scores = np.zeros((T, T), np.float32)
    for k in range(NCORES):
        scores[RPC * k:RPC * (k + 1), :] = res.results[k]["scores_t"].T
    scores += float(I["b2"][0])
    scores[np.arange(T), np.arange(T)] = 0.0
    return scores


# revision 16
# speedup vs baseline: 2.1838x; 2.1838x over previous
"""Trainium2 Bass kernel for the DependencyParseModel problem.

Pipeline (replicated biLSTM, pairwise scoring sharded over 8 cores):
  1. host: embedding gather, weight permute/pad into PE-friendly layouts.
     Contract dims are padded to 128 (FWL requires K=128; smaller contract
     disables fast weight load and quadruples the matvec cadence).
  2. device: XW = Wih @ x_aug (dense matmuls), 256-step LSTM scans with
     weights-stationary matvecs (bf16 FWL), both directions interleaved.
     Gate groups [i,f,g~] and [o] use separate full-bank PSUM groups so
     the sigmoid fires before the o-gate matvecs finish and each
     direction's pointwise chain hides under the other direction's
     matvec block (PSUM bank sharing between groups serializes the scan).
  3. device: pairwise-MLP scoring in transposed layout: bias-add on DVE,
     tanh on the scalar engine, W2 contraction on the PE.
  4. host: assemble [256,256], add b2, zero diagonal

Self-contained: hardcodes all shapes; no sibling imports.
"""

import os
import numpy as np
import ml_dtypes

T = 256
H = 400            # LSTM hidden
HP = 512           # padded hidden
G = 2048           # padded gate dim (4 gates x 512)
D1P = 1024         # padded layer-1 input dim (2 x HP)
MLP = 400
MLPP = 512         # padded MLP dim
NCORES = 8
RPC = T // NCORES  # rows per core (head-word rows)

BF16 = ml_dtypes.bfloat16

# gate blocks in OUR layout order [i, f, o, g~]; source ranges in torch order
_GATE_SRC = [(0, 400), (400, 800), (1200, 1600), (800, 1200)]

# m-chunk order for PSUM gate tiles: [i(0-3), f(4-7), g~(12-15), o(8-11)]
# so cols 0:12 = [i,f,g~] (one PSUM group) and 12:16 = [o] (second group).
MORDER = [0, 1, 2, 3, 4, 5, 6, 7, 12, 13, 14, 15, 8, 9, 10, 11]


def _permute_pad_gate_rows(W):
    """[1600, K] -> [2048, K]: torch gate order i,f,g,o -> blocks [i,f,o,g~], each padded to 512."""
    out = np.zeros((G, W.shape[1]), np.float32)
    for b, (s, e) in enumerate(_GATE_SRC):
        out[b * 512: b * 512 + (e - s)] = W[s:e]
    return out


def _permute_pad_gate_vec(v):
    out = np.zeros(G, np.float32)
    for b, (s, e) in enumerate(_GATE_SRC):
        out[b * 512: b * 512 + (e - s)] = v[s:e]
    return out


def _pad_cols(W, K):
    """[R, k] -> [R, K] zero-padded."""
    out = np.zeros((W.shape[0], K), np.float32)
    out[:, : W.shape[1]] = W
    return out


def _h_tile(v):
    """[400] -> [128, 4] tile, elem d -> (d % 128, d // 128)."""
    out = np.zeros(HP, np.float32)
    out[:H] = v
    return np.ascontiguousarray(out.reshape(4, 128).T)


_PROG_CACHE = {}


def _get_program(n_steps=T):
    key = ("prog", n_steps)
    if key in _PROG_CACHE:
        return _PROG_CACHE[key]

    import concourse.bass as bass
    import concourse.mybir as mybir
    import concourse.tile as tile
    from concourse import bacc

    F32 = mybir.dt.float32
    BF = mybir.dt.bfloat16
    AF = mybir.ActivationFunctionType
    ALU = mybir.AluOpType

    nc = bacc.Bacc("TRN2", target_bir_lowering=False, debug=False,
                   enable_asserts=False, num_devices=NCORES)

    # ---- DRAM I/O ----
    d_xT = nc.dram_tensor("xT", [HP, T], BF, kind="ExternalInput").ap()
    d_wihT0 = [nc.dram_tensor(f"wihT0{d}", [HP, G], BF, kind="ExternalInput").ap() for d in "fb"]
    d_wihT1 = [nc.dram_tensor(f"wihT1{d}", [D1P, G], BF, kind="ExternalInput").ap() for d in "fb"]
    d_whhT = [[nc.dram_tensor(f"whhT{l}{d}", [HP, G], BF, kind="ExternalInput").ap() for d in "fb"]
              for l in (0, 1)]
    d_h0 = [[nc.dram_tensor(f"h0_{l}{d}", [128, 4], BF, kind="ExternalInput").ap() for d in "fb"]
            for l in (0, 1)]
    d_c0 = [[nc.dram_tensor(f"c0_{l}{d}", [128, 4], F32, kind="ExternalInput").ap() for d in "fb"]
            for l in (0, 1)]
    d_w1aT = nc.dram_tensor("w1aT", [D1P, MLPP], BF, kind="ExternalInput").ap()
    d_w1bT = nc.dram_tensor("w1bT", [D1P, MLPP], BF, kind="ExternalInput").ap()
    d_skT = nc.dram_tensor("skT", [T, RPC], BF, kind="ExternalInput").ap()
    d_w2T = nc.dram_tensor("w2T", [128, 4], BF, kind="ExternalInput").ap()
    d_ident = nc.dram_tensor("ident", [128, 128], BF, kind="ExternalInput").ap()
    d_out = nc.dram_tensor("scores_t", [T, RPC], F32, kind="ExternalOutput").ap()

    with tile.TileContext(nc) as tc:
        from contextlib import ExitStack
        with ExitStack() as ctx:
            const = ctx.enter_context(tc.tile_pool(name="const", bufs=1))
            state = ctx.enter_context(tc.tile_pool(name="state", bufs=1))
            whhp = ctx.enter_context(tc.tile_pool(name="whhp", bufs=1))

            # --- constants / initial state in SBUF ---
            xT_sb = []
            for kc in range(4):
                xt = const.tile([128, T], BF, name=f"xT{kc}")
                nc.sync.dma_start(xt, d_xT[128 * kc:128 * (kc + 1), :])
                xT_sb.append(xt)
            h0_sb = [[None, None], [None, None]]
            c_sb = [[None, None], [None, None]]
            for l in (0, 1):
                for d in (0, 1):
                    t0 = const.tile([128, 4], BF, name=f"h0sb{l}{d}")
                    nc.sync.dma_start(t0, d_h0[l][d])
                    h0_sb[l][d] = t0
                    t1 = state.tile([128, 4], F32, name=f"csb{l}{d}")
                    nc.sync.dma_start(t1, d_c0[l][d])
                    c_sb[l][d] = t1
            whh_sb = [[None, None], [None, None]]
            for l in (0, 1):
                for d in (0, 1):
                    chunks = []
                    for kc in range(4):
                        w = whhp.tile([128, G], BF, name=f"whh{l}{d}{kc}")
                        nc.sync.dma_start(w, d_whhT[l][d][128 * kc:128 * (kc + 1), :])
                        chunks.append(w)
                    whh_sb[l][d] = chunks
            w1aT_sb, w1bT_sb = [], []
            for kc in range(8):
                wa = const.tile([128, MLPP], BF, name=f"w1aT{kc}")
                nc.sync.dma_start(wa, d_w1aT[128 * kc:128 * (kc + 1), :])
                w1aT_sb.append(wa)
                wb = const.tile([128, MLPP], BF, name=f"w1bT{kc}")
                nc.sync.dma_start(wb, d_w1bT[128 * kc:128 * (kc + 1), :])
                w1bT_sb.append(wb)
            skT_sb = []
            for kc in range(2):
                sk = const.tile([128, RPC], BF, name=f"skT{kc}")
                nc.sync.dma_start(sk, d_skT[128 * kc:128 * (kc + 1), :])
                skT_sb.append(sk)
            w2T_sb = const.tile([128, 4], BF, name="w2T")
            nc.sync.dma_start(w2T_sb, d_w2T)
            ident_sb = const.tile([128, 128], BF, name="ident")
            nc.sync.dma_start(ident_sb, d_ident)

            # persistent per-(layer,dir) h history [128, 4*T] bf16, col 4t+c = h_t[128c+p]
            hh_sb = [[state.tile([128, 4 * T], BF, name=f"hh{l}{d}") for d in (0, 1)]
                     for l in (0, 1)]
            # XW^T buffers, reused across layers: [128, 16*T] bf16, col mpos*T + t
            xwt_sb = [state.tile([128, 16 * T], BF, name=f"xwt{d}") for d in (0, 1)]

            def xwt_phase(layer, preloaded=None):
                """xwt_sb[d] <- Wih[layer][d] @ x_aug (all timesteps), m-chunks in MORDER."""
                K = 4 if layer == 0 else 8
                d_wih = d_wihT0 if layer == 0 else d_wihT1
                with tc.tile_pool(name=f"wihp{layer}", bufs=1) as wp, \
                     tc.tile_pool(name=f"xwps{layer}", bufs=4, space="PSUM") as pp:
                    for d in (0, 1):
                        if preloaded is not None:
                            wih_sb = preloaded[d]
                        else:
                            wih_sb = []
                            for kc in range(K):
                                w = wp.tile([128, G], BF, name=f"wih{layer}{d}{kc}",
                                            tag=f"wih{kc}")
                                nc.sync.dma_start(w, d_wih[d][128 * kc:128 * (kc + 1), :])
                                wih_sb.append(w)
                        if layer == 0:
                            rhs = xT_sb
                        else:
                            rhs = []
                            for kc in range(K):
                                hhr = hh_sb[0][kc // 4][:].rearrange(
                                    "p (t c) -> p c t", c=4)
                                rhs.append(hhr[:, kc % 4, :])
                        for mpos in range(16):
                            m = MORDER[mpos]
                            ps = pp.tile([128, T], F32, name=f"xwps{layer}{d}{mpos}",
                                         tag="xwps")
                            for kc in range(K):
                                nc.tensor.matmul(
                                    ps, wih_sb[kc][:, 128 * m:128 * (m + 1)], rhs[kc],
                                    start=(kc == 0), stop=(kc == K - 1))
                            nc.vector.tensor_copy(
                                xwt_sb[d][:, T * mpos:T * (mpos + 1)], ps)

            def scan_phase(layer):
                # gate layout (permuted on host, m-chunks reordered per MORDER):
                # PSUM g_ifg cols 0:4=i, 4:8=f, 8:12=g~ ; g_o cols 0:4=o.
                # g~ pre-activations DOUBLED (host scaled the weights), so
                # sigmoid gives tanh(x) = 2*(sigmoid(2x) - 0.5).
                with tc.tile_pool(name=f"psg{layer}", bufs=2, space="PSUM") as p_g, \
                     tc.tile_pool(name=f"sg{layer}", bufs=3) as sgp:
                    for s in range(n_steps):
                        for d in (0, 1):
                            t = s if d == 0 else T - 1 - s
                            hh = hh_sb[layer][d]
                            if s == 0:
                                h_prev = h0_sb[layer][d]
                            else:
                                tp = t - 1 if d == 0 else t + 1
                                h_prev = hh[:, 4 * tp:4 * tp + 4]
                            xwr = xwt_sb[d][:].rearrange("p (m t) -> p m t", t=T)
                            # each accumulation group gets a full 2KB PSUM bank
                            # (bank sharing between open groups serializes)
                            g_ifg = p_g.tile([128, 12], F32, name=f"gifg{d}",
                                             tag=f"gifg{d}", padded_shape=[128, 512])
                            g_o = p_g.tile([128, 4], F32, name=f"go{d}",
                                           tag=f"go{d}", padded_shape=[128, 512])
                            # seed PSUM with XW[t] via identity matmul (start),
                            # then accumulate the matvec tiles on top
                            nc.tensor.matmul(g_ifg, ident_sb, xwr[:, 0:12, t],
                                             start=True, stop=False,
                                             skip_group_check=True)
                            for mpos in range(12):
                                m = MORDER[mpos]
                                for kc in range(4):
                                    nc.tensor.matmul(
                                        g_ifg[:, mpos:mpos + 1],
                                        whh_sb[layer][d][kc][:, 128 * m:128 * (m + 1)],
                                        h_prev[:, kc:kc + 1],
                                        start=False,
                                        stop=(mpos == 11 and kc == 3),
                                        skip_group_check=True)
                            nc.tensor.matmul(g_o, ident_sb, xwr[:, 12:16, t],
                                             start=True, stop=False,
                                             skip_group_check=True)
                            for mpos in range(12, 16):
                                m = MORDER[mpos]
                                for kc in range(4):
                                    nc.tensor.matmul(
                                        g_o[:, mpos - 12:mpos - 11],
                                        whh_sb[layer][d][kc][:, 128 * m:128 * (m + 1)],
                                        h_prev[:, kc:kc + 1],
                                        start=False,
                                        stop=(mpos == 15 and kc == 3),
                                        skip_group_check=True)
                            S = sgp.tile([128, 12], F32, name=f"S{d}", tag=f"S{d}")
                            So = sgp.tile([128, 4], F32, name=f"So{d}", tag=f"So{d}")
                            nc.scalar.activation(S, g_ifg, AF.Sigmoid)
                            nc.scalar.activation(So, g_o, AF.Sigmoid)
                            cc = c_sb[layer][d]
                            t1 = sgp.tile([128, 4], F32, name=f"t1{d}", tag=f"t1{d}")
                            u2 = sgp.tile([128, 4], F32, name=f"u2{d}", tag=f"u2{d}")
                            # c = sig(f)*c + sig(i)*2*(S_g~ - 0.5)
                            nc.vector.tensor_mul(t1, S[:, 4:8], cc)
                            nc.vector.scalar_tensor_tensor(
                                u2, S[:, 8:12], -0.5, S[:, 0:4],
                                op0=ALU.add, op1=ALU.mult)
                            nc.vector.scalar_tensor_tensor(
                                cc, u2, 2.0, t1,
                                op0=ALU.mult, op1=ALU.add)
                            tct = sgp.tile([128, 4], F32, name=f"tc{d}", tag=f"tc{d}")
                            nc.scalar.activation(tct, cc, AF.Tanh)
                            nc.vector.tensor_mul(hh[:, 4 * t:4 * t + 4], So, tct)

            xwt_phase(0)
            # prefetch layer-1 Wih during the L0 scan (DMAs have no deps on
            # the scan, so the scheduler overlaps them with it)
            wih1_pre = [[], []]
            with tc.tile_pool(name="wihpre1", bufs=1) as wpre:
                for d in (0, 1):
                    for kc in range(8):
                        w = wpre.tile([128, G], BF, name=f"wihpre{d}{kc}")
                        nc.sync.dma_start(w, d_wihT1[d][128 * kc:128 * (kc + 1), :])
                        wih1_pre[d].append(w)
                scan_phase(0)
                # ones row for layer-1 bias trick: x1 dim 416 -> (c=3, p=32) of
                # fwd hist (DVE start partition must be 32-aligned; 416 is pad)
                hh0f_r = hh_sb[0][0][:].rearrange("p (t c) -> p c t", c=4)
                nc.vector.memset(hh0f_r[32:33, 3, :], 1.0)
                xwt_phase(1, preloaded=wih1_pre)
            scan_phase(1)
            hh1f_r = hh_sb[1][0][:].rearrange("p (t c) -> p c t", c=4)
            nc.vector.memset(hh1f_r[32:33, 3, :], 1.0)

            # ---------- pairwise scoring (transposed layout) ----------
            def hvecT_chunk(kc, jt):
                """lhsT [128, 128]: hvec.T rows [128kc:128kc+128], cols [128jt:+128]."""
                hhr = hh_sb[1][kc // 4][:].rearrange("p (t c) -> p c t", c=4)
                return hhr[:, kc % 4, 128 * jt:128 * (jt + 1)]

            def hvecT_full(kc):
                hhr = hh_sb[1][kc // 4][:].rearrange("p (t c) -> p c t", c=4)
                return hhr[:, kc % 4, :]

            with tc.tile_pool(name="pw", bufs=1) as pw:
                pjT_sb, piT_loc = [], []
                with tc.tile_pool(name="pwps", bufs=2, space="PSUM") as pwps:
                    # pjT[mc] = (W1b @ hvec.T) chunk: [128 mlp, 256 j]
                    for mc in range(4):
                        ps = pwps.tile([128, T], F32, name=f"pjTps{mc}", tag="projps")
                        for kc in range(8):
                            nc.tensor.matmul(ps,
                                             w1bT_sb[kc][:, 128 * mc:128 * (mc + 1)],
                                             hvecT_full(kc),
                                             start=(kc == 0), stop=(kc == 7))
                        pj = pw.tile([128, T], BF, name=f"pjT{mc}")
                        nc.vector.tensor_copy(pj, ps)
                        pjT_sb.append(pj)
                    # pi rows: [128 i, 512 mlp] x2, then select this core's 32
                    pi_sb = []
                    for jt in range(2):
                        ps = pwps.tile([128, MLPP], F32, name=f"pips{jt}",
                                       tag="projps2")
                        for kc in range(8):
                            nc.tensor.matmul(ps, hvecT_chunk(kc, jt), w1aT_sb[kc],
                                             start=(kc == 0), stop=(kc == 7))
                        pi = pw.tile([128, MLPP], BF, name=f"pi{jt}")
                        nc.vector.tensor_copy(pi, ps)
                        pi_sb.append(pi)
                    ps = pwps.tile([RPC, MLPP], F32, name="pikps", tag="projps2")
                    for kc in range(2):
                        nc.tensor.matmul(ps, skT_sb[kc], pi_sb[kc],
                                         start=(kc == 0), stop=(kc == 1))
                    pik = pw.tile([RPC, MLPP], BF, name="pik")
                    nc.vector.tensor_copy(pik, ps)
                    # transpose pik -> piT_loc[mc]: [128 mlp, 32 rows] f32 sbuf
                    for mc in range(4):
                        tps = pwps.tile([128, RPC], BF, name=f"piTps{mc}",
                                        tag="projps")
                        nc.tensor.transpose(tps, pik[:, 128 * mc:128 * (mc + 1)],
                                            ident_sb[0:RPC, 0:RPC])
                        tl = pw.tile([128, RPC], F32, name=f"piT{mc}")
                        nc.vector.tensor_copy(tl, tps)
                        piT_loc.append(tl)

                with tc.tile_pool(name="bps", bufs=1, space="PSUM") as bps, \
                     tc.tile_pool(name="bsb", bufs=3) as bsb:
                    scoresT_ps = [bps.tile([128, RPC], F32, name=f"scT{jh}",
                                           tag=f"scT{jh}") for jh in range(2)]
                    for g in range(8):          # 4 head rows per group
                        Tas = []
                        for mc in range(4):
                            Bt = bsb.tile([128, 4 * T], BF, name=f"B{g}{mc}",
                                          tag="B")
                            for q in range(4):
                                r = 4 * g + q
                                nc.vector.tensor_scalar_add(
                                    Bt[:, T * q:T * (q + 1)], pjT_sb[mc],
                                    piT_loc[mc][:, r:r + 1])
                            Ta = bsb.tile([128, 4 * T], BF, name=f"Ta{g}{mc}",
                                          tag=f"Ta{mc}", bufs=2)
                            nc.scalar.activation(Ta, Bt, AF.Tanh)
                            Tas.append(Ta)
                        # each psum column's 4-matmul accumulation group is
                        # consecutive (interleaved per-column groups misaccumulate)
                        for q in range(4):
                            r = 4 * g + q
                            for jh in range(2):
                                for mc in range(4):
                                    nc.tensor.matmul(
                                        scoresT_ps[jh][:, r:r + 1],
                                        Tas[mc][:, T * q + 128 * jh:T * q + 128 * (jh + 1)],
                                        w2T_sb[:, mc:mc + 1],
                                        start=(mc == 0), stop=(mc == 3),
                                        skip_group_check=True)
                    for jh in range(2):
                        sc = pw.tile([128, RPC], F32, name=f"scsb{jh}")
                        nc.vector.tensor_copy(sc, scoresT_ps[jh])
                        nc.sync.dma_start(d_out[128 * jh:128 * (jh + 1), :], sc)

    nc.compile()
    _PROG_CACHE[key] = nc
    return nc


def _prep_inputs(inputs):
    """Host-side prep: gather embeddings, build padded/permuted device tensors."""
    I = {k: np.asarray(v) for k, v in inputs.items()}
    x = np.concatenate([I["word_emb"][I["words"]], I["tag_emb"][I["tags"]]],
                       axis=1).astype(np.float32)          # [T, 400]
    xT = np.zeros((HP, T), np.float32)
    xT[:H] = x.T
    xT[H] = 1.0                                            # bias row

    common = {"xT": xT.astype(BF16)}
    for l in (0, 1):
        Din = H if l == 0 else 2 * H
        DinP = HP if l == 0 else D1P
        for di, d in enumerate("fb"):
            wih = _permute_pad_gate_rows(I[f"Wih{l}"][di])  # [2048, Din]
            if l == 0:
                wihp = _pad_cols(wih, HP)                   # [2048, 512]
            else:
                wihp = np.zeros((G, D1P), np.float32)
                wihp[:, :H] = wih[:, :H]                    # fwd part
                wihp[:, HP:HP + H] = wih[:, H:2 * H]        # bwd part
            bias = _permute_pad_gate_vec(I[f"bih{l}"][di] + I[f"bhh{l}"][di])
            # bias column: layer 0's ones-row is xT row 400; layer 1's is the
            # hist pad position 416 (partition-32-aligned for the memset)
            wihp[:, H if l == 0 else 416] += bias
            wihp[1536:] *= 2.0      # g~ block doubled: tanh(x) = 2*sig(2x)-1
            common[f"wihT{l}{d}"] = np.ascontiguousarray(wihp.T).astype(BF16)

            whh = _pad_cols(_permute_pad_gate_rows(I[f"Whh{l}"][di]), HP)
            whh[1536:] *= 2.0
            common[f"whhT{l}{d}"] = np.ascontiguousarray(whh.T).astype(BF16)

            common[f"h0_{l}{d}"] = _h_tile(I["h0"][l, di]).astype(BF16)
            common[f"c0_{l}{d}"] = _h_tile(I["c0"][l, di]).astype(np.float32)

    W1 = I["W1"].astype(np.float32)                         # [400, 1600]
    W1a, W1b = W1[:, :2 * H], W1[:, 2 * H:]                 # [400, 800] each

    def mlp_T(W, bias=None):
        Wp = np.zeros((MLPP, D1P), np.float32)
        Wp[:MLP, :H] = W[:, :H]
        Wp[:MLP, HP:HP + H] = W[:, H:]
        if bias is not None:
            Wp[:MLP, 416] += bias                           # hvec ones-row at 416
        return np.ascontiguousarray(Wp.T).astype(BF16)      # [1024, 512]

    common["w1aT"] = mlp_T(W1a, I["b1"].astype(np.float32))
    common["w1bT"] = mlp_T(W1b)
    w2T = np.zeros((128, 4), np.float32)
    w2flat = np.zeros(MLPP, np.float32)
    w2flat[:MLP] = I["W2"][0]
    for mc in range(4):
        w2T[:, mc] = w2flat[128 * mc:128 * (mc + 1)]
    common["w2T"] = w2T.astype(BF16)
    common["ident"] = np.eye(128, dtype=np.float32).astype(BF16)

    in_maps = []
    for k in range(NCORES):
        m = dict(common)
        sk = np.zeros((T, RPC), np.float32)
        sk[RPC * k + np.arange(RPC), np.arange(RPC)] = 1.0
        m["skT"] = sk.astype(BF16)
        in_maps.append(m)
    return in_maps, I


def _ensure_ntff_hook():
    """Shim antenv.axon_hooks (absent in this image) so trace=True works."""
    import sys
    import types
    import antenv
    if hasattr(antenv, "axon_hooks") or "antenv.axon_hooks" in sys.modules:
        return
    hook = None
    try:
        from trn_agent_boot.trn_boot import _ntff_profile_via_ctypes
        hook = _ntff_profile_via_ctypes("/opt/axon/libaxon_pjrt.so")
    except Exception:
        hook = None
    mod = types.ModuleType("antenv.axon_hooks")
    state = {"hook": hook}
    mod.get_axon_ntff_profile_hook = lambda: state["hook"]
    mod.set_axon_ntff_profile_hook = lambda h: state.update(hook=h)
    sys.modules["antenv.axon_hooks"] = mod
    antenv.axon_hooks = mod


def kernel(**inputs):
    from concourse import bass_utils
    from concourse.bass_interp import get_hw_module

    nc = _get_program()
    in_maps, I = _prep_inputs(inputs)

    trace = bool(int(os.environ.get("KERNEL_TRACE", "0")))
    if trace:
        _ensure_ntff_hook()
    old_m = nc.m
    nc.m = get_hw_module(nc.m)
    try:
        res = bass_utils.run_bass_kernel_spmd(
            nc, in_maps, core_ids=list(range(NCORES)), trace=trace)
    finally:
        nc.m = old_m
    if trace and res.exec_time_ns is not None:
        print(f"HW exec time: {res.exec_time_ns} ns")
        kernel.last_exec_time_ns = res.exec_time_ns

    scores = np.zeros((T, T), np.float32)
    for k in range(NCORES):
        scores[RPC * k:RPC * (k + 1), :] = res.results[k]["scores_t"].T
    scores += float(I["b2"][0])
    scores[np.arange(T), np.arange(T)] = 0.0
    return scores
